# revision 28
# baseline (speedup 1.0000x reference)
"""3-layer GCN (PyG GCNConv semantics) on 8 Trainium2 NeuronCores via Bass.

Sharding (per the hint): nodes sharded across 8 cores, edges partitioned
by destination node, [128,128] weights replicated, source features
halo-exchanged (AllGather) per layer.

Weights are separable (w_edge = dinv[src]*dinv[dst]), so the kernel
stores dinv[src]-scaled features in the halo tables, aggregates with
exact {0,1} one-hot matrices held in fp8, and applies dinv[dst] as a
per-column post-scale. With zero biases (the graded case) the dst scale
is deferred through the next layer's LeakyReLU (positively homogeneous):
the host pre-divides x by dinv, per-layer table scaling becomes dinv^2,
and only the final layer post-scales. The self-loop contribution is an identity-matmul
of the on-chip h shard fused into phase A.

Per layer:
  A) groups of 4 tiles: act = Lrelu(x^T), h = act.T @ W (PE, PSUM
     [128,512]), hsall = h * dinv_src (one DVE mult per group), DMA the
     group to its DRAM slice (AllGather input), and the fused self pass:
     h tile @ identity -> xnxt (PE transpose + DVE copy).
  B) 4 pipelined AllGathers, one per shard slice -> 4 quarter tables
     [NPAD/4, 128] fp16 on every core (dma_gather indices are int16, so
     each quarter table stays < 32768 rows).
  C) per quarter q (as soon as table_q lands), chunks of 4 tiles:
     one 1024-idx dma_gather (single_packet, round-robin over 4 SWDGE
     queues - gathers are HBM-latency bound, so 4 queues ~4x the
     random-read throughput and ~12 msg buffers keep them all fed),
     a [128,1024] fp8 one-hot S^T load, 8 PE matmuls accumulating
     out^T[feat,dst] += M^T S^T in a [128,512] PSUM tile, one DVE add
     into xnxt; after the last quarter, post-scale by dinv_dst and add
     the bias.

Host-side packing: nodes -> tiles balancing in-degree, tiles -> 4
quarters keeping per-(dst tile, quarter) counts <= qb*128, quarter
tiles -> cores balancing per-core load. Pad gather slots repeat the
segment's last valid row (HBM row-buffer hit) and carry weight 0.
x lives on-chip in fp16 (two [128, 12800] ping-pong buffers).
"""

import heapq

import ml_dtypes
import numpy as np

import concourse.bacc as bacc
import concourse.bass as bass
import concourse.mybir as mybir
import concourse.tile as tile
from concourse.bass_utils import run_bass_kernel_spmd

N_CORES = 8
N_QUART = 4
D = 128
P = 128


class Cfg:
    def __init__(self, n_nodes, n_edges, tiles_per_core, qblocks,
                 chunk_tiles, fp16=True):
        assert tiles_per_core % N_QUART == 0
        self.n_nodes = n_nodes
        self.n_edges = n_edges
        self.tpc = tiles_per_core
        self.tpq = tiles_per_core // N_QUART   # tiles per (core, quarter)
        self.qb = qblocks                 # 128-blocks per (tile, quarter)
        self.chunk = chunk_tiles
        self.shard = tiles_per_core * P
        self.slice_rows = self.shard // N_QUART    # h shard slice rows
        self.npad = self.shard * N_CORES
        self.qrows = self.npad // N_QUART          # quarter table rows
        self.ngmsg = self.tpc * N_QUART * qblocks * P
        self.fp16 = fp16
        assert self.npad >= n_nodes
        assert self.qrows <= 32768
        # dma_gather ring limit handled by splitting into <=1024-idx
        # instructions in build_program
        self.n_tiles = N_CORES * tiles_per_core


FULL = Cfg(n_nodes=100000, n_edges=640000, tiles_per_core=100,
           qblocks=2, chunk_tiles=4)

# ------------------------------------------------------------- host prep


def _pack_nodes_to_tiles(deg, cfg):
    """Nodes -> anonymous tiles (128 each), balancing total in-degree."""
    n_tiles = cfg.n_tiles
    order = np.argsort(-deg, kind="stable")
    heap = [(0, t) for t in range(n_tiles)]
    heapq.heapify(heap)
    counts = np.zeros(n_tiles, dtype=np.int64)
    node_tile = np.empty(cfg.n_nodes, dtype=np.int64)
    node_slot = np.empty(cfg.n_nodes, dtype=np.int64)
    for n in order:
        load, t = heapq.heappop(heap)
        node_tile[n] = t
        node_slot[n] = counts[t]
        counts[t] += 1
        if counts[t] < P:
            heapq.heappush(heap, (load + int(deg[n]), t))
    return node_tile, node_slot


def _try_pack_quarters(M, cfg, order, soft_margin):
    n_tiles = cfg.n_tiles
    per_q = n_tiles // N_QUART
    soft = cfg.qb * P - soft_margin
    R = np.zeros((N_QUART, n_tiles), dtype=np.float64)
    sizes = np.zeros(N_QUART, dtype=np.int64)
    tile_quarter = np.full(n_tiles, -1, dtype=np.int64)
    for s in order:
        row = M[s]
        best_q, best_pen = -1, None
        for q in range(N_QUART):
            if sizes[q] >= per_q:
                continue
            nr = R[q] + row
            over = np.maximum(nr - soft, 0)
            pen = (float((over * over).sum()), float(nr.max()),
                   float(sizes[q]))
            if best_pen is None or pen < best_pen:
                best_q, best_pen = q, pen
        R[best_q] += row
        sizes[best_q] += 1
        tile_quarter[s] = best_q
    return tile_quarter, R.max()


def _repair_quarters(M, tq, cfg, rng, iters=4000):
    """Local-search swap repair: drive per-(dst tile, quarter) counts
    under the hard cap by swapping tiles between quarters."""
    cap = cfg.qb * P
    n_tiles = cfg.n_tiles
    Mf = M.astype(np.float64)
    R = np.zeros((N_QUART, n_tiles), dtype=np.float64)
    for s in range(n_tiles):
        R[tq[s]] += Mf[s]
    members = [list(np.where(tq == q)[0]) for q in range(N_QUART)]

    def viol(r):
        o = np.maximum(r - cap, 0)
        return (o * o).sum(axis=-1)

    stall = 0
    for _ in range(iters):
        if (R <= cap).all():
            return tq, 0.0
        q1, d = np.unravel_index(np.argmax(R - cap), R.shape)
        mem1 = members[q1]
        contrib = M[mem1, d]
        ncand = 4 if stall < 50 else 8
        cand1 = [mem1[i] for i in np.argsort(-contrib)[:ncand]]
        base1 = float(viol(R[q1]))
        best = None
        for s1 in cand1:
            r1_wo = R[q1] - Mf[s1]
            for q2 in range(N_QUART):
                if q2 == q1:
                    continue
                mem2 = np.asarray(members[q2])
                base2 = float(viol(R[q2]))
                nr1 = r1_wo[None, :] + Mf[mem2]
                nr2 = (R[q2] + Mf[s1])[None, :] - Mf[mem2]
                delta = viol(nr1) + viol(nr2) - base1 - base2
                i = int(np.argmin(delta))
                if best is None or delta[i] < best[0]:
                    best = (float(delta[i]), s1, int(mem2[i]), q1, q2)
        if best is None or best[0] >= -1e-9:
            stall += 1
            if stall > 100:
                break
            q2 = int(rng.integers(0, N_QUART - 1))
            q2 = q2 if q2 < q1 else q2 + 1
            s1 = members[q1][int(rng.integers(len(members[q1])))]
            s2 = members[q2][int(rng.integers(len(members[q2])))]
            best = (0.0, s1, s2, q1, q2)
        else:
            stall = 0
        _, s1, s2, qa, qb_ = best
        R[qa] += Mf[s2] - Mf[s1]
        R[qb_] += Mf[s1] - Mf[s2]
        members[qa].remove(s1)
        members[qa].append(s2)
        members[qb_].remove(s2)
        members[qb_].append(s1)
        tq[s1], tq[s2] = qb_, qa
    if (R <= cap).all():
        return tq, 0.0
    return tq, float(max(viol(R[q]) for q in range(N_QUART)))


def _pack_tiles_to_quarters(M, cfg):
    hard_cap = cfg.qb * P
    rng = np.random.default_rng(1234)
    attempts = [np.argsort(-M.sum(axis=1), kind="stable"),
                rng.permutation(cfg.n_tiles)]
    best_max = None
    for order in attempts:
        tq, rmax = _try_pack_quarters(M, cfg, order, 16)
        if rmax <= hard_cap:
            return tq
        tq, vmax = _repair_quarters(M, tq, cfg, rng)
        if vmax == 0:
            return tq
        if best_max is None or vmax < best_max:
            best_max = vmax
    raise RuntimeError(f"quarter packing failed: viol {best_max}")


def prepare(x, edge_index, cfg, fold=True):
    n = cfg.n_nodes
    src = np.asarray(edge_index[0], dtype=np.int64)
    dst = np.asarray(edge_index[1], dtype=np.int64)
    deg = (np.bincount(dst, minlength=n) + 1).astype(np.float64)
    dinv = 1.0 / np.sqrt(deg)

    node_tile, node_slot = _pack_nodes_to_tiles(deg, cfg)

    stile = node_tile[src]
    dtile = node_tile[dst]
    n_tiles = cfg.n_tiles
    M = np.zeros((n_tiles, n_tiles), dtype=np.int64)
    np.add.at(M, (stile, dtile), 1)

    tile_quarter = _pack_tiles_to_quarters(M, cfg)

    # quarter tiles -> (core, local quarter slot): balance per-core load
    tile_pos = np.empty(n_tiles, dtype=np.int64)
    tload = M.sum(axis=0)
    core_load = np.zeros(N_CORES, dtype=np.int64)
    for q in range(N_QUART):
        tiles_q = np.where(tile_quarter == q)[0]
        tiles_q = tiles_q[np.argsort(-tload[tiles_q], kind="stable")]
        fill = np.zeros(N_CORES, dtype=np.int64)
        for t in tiles_q:
            avail = np.where(fill < cfg.tpq)[0]
            k = avail[np.argmin(core_load[avail])]
            tile_pos[t] = k * cfg.tpc + q * cfg.tpq + fill[k]
            fill[k] += 1
            core_load[k] += tload[t]

    row_id = tile_pos[node_tile] * P + node_slot

    # quarter-table row of a node (node's slice concatenated per core)
    lrow = row_id % cfg.shard
    core_of = row_id // cfg.shard
    q_of = lrow // cfg.slice_rows
    qtab_row = core_of * cfg.slice_rows + (lrow - q_of * cfg.slice_rows)

    # ---- per-edge message placement
    drow_tile = tile_pos[node_tile[dst]]
    dslot = node_slot[dst]
    squart = q_of[src]
    srow_qt = qtab_row[src]

    seg = drow_tile * N_QUART + squart
    # secondary sort by gather row for HBM locality within a segment
    seg_order = np.lexsort((srow_qt, seg))
    seg_s = seg[seg_order]
    seg_cnt = np.bincount(seg_s, minlength=n_tiles * N_QUART)
    seg_cap = cfg.qb * P
    if seg_cnt.max() > seg_cap:
        raise RuntimeError(f"segment overflow {seg_cnt.max()} > {seg_cap}")
    seg_start = np.concatenate([[0], np.cumsum(seg_cnt)])
    within = np.arange(len(seg_s)) - seg_start[seg_s]

    pslot = seg_s * seg_cap + within
    m_idx = np.zeros(n_tiles * N_QUART * seg_cap, dtype=np.int64)
    m_w = np.zeros(n_tiles * N_QUART * seg_cap, dtype=np.float32)
    m_dslot = np.zeros(n_tiles * N_QUART * seg_cap, dtype=np.int64)
    # pad slots repeat the segment's last valid row (likely still in the
    # HBM row buffer) rather than hammering row 0
    pad_mask = np.ones(n_tiles * N_QUART * seg_cap, dtype=bool)
    pad_mask[pslot] = False
    have = seg_cnt > 0
    seg_last = np.zeros(n_tiles * N_QUART, dtype=np.int64)
    seg_last[have] = srow_qt[seg_order][
        np.minimum(seg_start[1:][have] - 1, len(seg_order) - 1)]
    src_fill = np.repeat(seg_last, seg_cap)
    m_idx[pad_mask] = src_fill[pad_mask]
    m_idx[pslot] = srow_qt[seg_order]
    m_w[pslot] = 1.0
    m_dslot[pslot] = dslot[seg_order]

    m_idx = m_idx.reshape(n_tiles, N_QUART, cfg.qb, P)
    m_w = m_w.reshape(n_tiles, N_QUART, cfg.qb, P)
    m_dslot = m_dslot.reshape(n_tiles, N_QUART, cfg.qb, P)

    xTp = np.zeros((D, cfg.npad), dtype=np.float32)
    xv = np.asarray(x, dtype=np.float32)
    if fold:
        # uniform deferred-dinv form: layer tables scale by dinv^2, so
        # pre-divide the input (x has no deferred factor yet)
        xv = xv / dinv.astype(np.float32)[:, None]
    xTp[:, row_id] = xv.T

    dinv_row = np.zeros(cfg.npad, dtype=np.float32)
    dinv_row[row_id] = dinv.astype(np.float32)

    chunks = []
    t0 = 0
    while t0 < cfg.tpc:
        chunks.append((t0, min(cfg.chunk, cfg.tpc - t0)))
        t0 += cfg.chunk

    sdt = np.float16 if cfg.fp16 else np.float32
    per_core = []
    for k in range(N_CORES):
        # gather-block order: pass-major (q), then chunk, tile, qb
        gblocks = []
        for q in range(N_QUART):
            for (t0, nt) in chunks:
                for ti in range(nt):
                    for qb_i in range(cfg.qb):
                        gblocks.append((k * cfg.tpc + t0 + ti, q, qb_i))
        gb = np.asarray(gblocks, dtype=np.int64)
        pos, qq, qb_i = gb[:, 0], gb[:, 1], gb[:, 2]
        blk_idx = m_idx[pos, qq, qb_i]             # [ngblk, 128]
        blk_w = m_w[pos, qq, qb_i]
        blk_dslot = m_dslot[pos, qq, qb_i]

        midx = blk_idx.reshape(-1)
        ncols = cfg.ngmsg // 16
        idx16 = np.empty((16, ncols), dtype=np.int16)
        ar = np.arange(cfg.ngmsg)
        idx16[ar % 16, ar // 16] = midx.astype(np.int16)
        idx16 = np.tile(idx16, (8, 1))

        # S^T blocks: {0,1} one-hot gather blocks + one identity (fp8)
        ngblk = len(gblocks)
        sts = np.zeros((P, (ngblk + 1) * P), dtype=np.float32)
        bcol = (np.arange(ngblk)[:, None] * P + blk_dslot)
        prow = np.tile(np.arange(P)[None, :], (ngblk, 1))
        sts[prow.ravel(), bcol.ravel()] = blk_w.ravel()
        sts[np.arange(P), ngblk * P + np.arange(P)] = 1.0

        shard_cols = np.arange(k * cfg.shard, (k + 1) * cfg.shard)
        dinv_tile = dinv_row[shard_cols].reshape(cfg.tpc, P)  # [tpc, P]
        if fold:
            # dinv[dst] deferred through the next layer's LeakyReLU
            # (positively homogeneous): table scale becomes dinv^2
            dinv_tile = dinv_tile * dinv_tile
        # [P(slot), tpc*D]: column t*D+f holds dinv(slot-node of tile t)
        hdinv = np.repeat(dinv_tile[:, :, None], D, axis=2)  # [tpc,P,D]
        hdinv = hdinv.transpose(1, 0, 2).reshape(P, cfg.tpc * D)
        xdinv = np.tile(dinv_row[shard_cols][None, :], (P, 1))

        per_core.append({
            "xT": np.ascontiguousarray(
                xTp[:, k * cfg.shard:(k + 1) * cfg.shard]).astype(sdt),
            "idx16": idx16,
            "sts": sts.astype(ml_dtypes.float8_e4m3fn),
            "hdinv": np.ascontiguousarray(hdinv, dtype=sdt),
            "xdinv": xdinv.astype(sdt),
        })
    return per_core, row_id


# ------------------------------------------------------------ bass build

_FP = mybir.dt.float32


def build_program(cfg, ablate=(), repeats=1, nqueues=4, ag=4,
                  hpsb=2, opsb=2, mbufs=12, qopsb=2,
                  chunk=None, fold=True):
    """ablate: subset of {"noA","noB","noC","nogather"} for timing
    experiments (results wrong). repeats: timing instrument."""
    nc = bacc.Bacc("TRN2", target_bir_lowering=False, debug=False,
                   num_devices=N_CORES, num_swdge_queues=nqueues)
    gq = [0]
    if chunk is not None:
        import copy as _copy
        cfg = _copy.copy(cfg)
        cfg.chunk = chunk
    _HD = mybir.dt.float16 if cfg.fp16 else _FP
    xT_in = nc.declare_dram_parameter("xT", [D, cfg.shard], _HD,
                                      isOutput=False)
    idx_in = nc.declare_dram_parameter("idx16", [P, cfg.ngmsg // 16],
                                       mybir.dt.int16, isOutput=False)
    _F8 = mybir.dt.float8e4
    nst = cfg.ngmsg + P
    sts_in = nc.declare_dram_parameter("sts", [P, nst], _F8,
                                       isOutput=False)
    dinv_in = nc.declare_dram_parameter("hdinv", [P, cfg.tpc * D], _HD,
                                        isOutput=False)
    xdinv_in = nc.declare_dram_parameter("xdinv", [P, cfg.shard], _HD,
                                         isOutput=False)
    w_ins = [nc.declare_dram_parameter(f"W{i}", [D, D], _FP, isOutput=False)
             for i in range(3)]
    b_ins = [nc.declare_dram_parameter(f"b{i}", [D, 1], _FP, isOutput=False)
             for i in range(3)]
    out_dram = nc.declare_dram_parameter("out", [D, cfg.shard], _HD,
                                         isOutput=True)

    h_slices = [nc.dram_tensor(f"h_sl{q}", [cfg.slice_rows, D], _HD)
                for q in range(N_QUART)]
    tables = [nc.dram_tensor(f"table{q}", [cfg.qrows, D], _HD,
                             addr_space="Shared")
              for q in range(N_QUART)]

    groups = [list(range(N_CORES))]

    chunks = []
    t0 = 0
    while t0 < cfg.tpc:
        chunks.append((t0, min(cfg.chunk, cfg.tpc - t0)))
        t0 += cfg.chunk

    gcols = cfg.chunk * cfg.qb * P      # msg cols per (chunk, quarter)

    with tile.TileContext(nc, num_cores=N_CORES) as tc:
        with (
            tc.tile_pool(name="const", bufs=1) as cpool,
            tc.tile_pool(name="actp", bufs=3) as actp,
            tc.tile_pool(name="hps", bufs=hpsb, space="PSUM") as hps,
            tc.tile_pool(name="msgp", bufs=mbufs) as msgp,
            tc.tile_pool(name="stp", bufs=mbufs) as stp,
            tc.tile_pool(name="ops", bufs=opsb, space="PSUM") as ops,
            tc.tile_pool(name="qops", bufs=qopsb, space="PSUM") as qops,
        ):
            xa = cpool.tile([D, cfg.shard], _HD, tag="xa")
            xb = cpool.tile([D, cfg.shard], _HD, tag="xb")
            idxt = cpool.tile([P, cfg.ngmsg // 16], mybir.dt.int16,
                              tag="idxt")
            wts = [cpool.tile([D, D], _FP, tag=f"w{i}", name=f"w{i}")
                   for i in range(3)]
            bts = [cpool.tile([D, 1], _FP, tag=f"b{i}", name=f"b{i}")
                   for i in range(3)]
            # persistent on-chip h shard (node-major per tile), identity
            # self block, per-slot dinv, per-column dinv broadcast
            hsall = cpool.tile([P, cfg.tpc * D], _HD, tag="hsall")
            ident = cpool.tile([P, P], _F8, tag="ident")
            hdinv = cpool.tile([P, cfg.tpc * D], _HD, tag="hdinv")
            xdinv = cpool.tile([P, cfg.shard], _HD, tag="xdinv")

            nc.sync.dma_start(out=xa[:], in_=xT_in[:])
            nc.sync.dma_start(out=idxt[:], in_=idx_in[:])
            nc.sync.dma_start(out=ident[:],
                              in_=sts_in[:, cfg.ngmsg:cfg.ngmsg + P])
            nc.sync.dma_start(out=hdinv[:], in_=dinv_in[:])
            nc.sync.dma_start(out=xdinv[:], in_=xdinv_in[:])
            for i in range(3):
                nc.sync.dma_start(out=wts[i][:], in_=w_ins[i][:])
                nc.sync.dma_start(out=bts[i][:], in_=b_ins[i][:])

            # phase-A groups of <=8 tiles, never crossing a slice boundary
            agroups = []
            for sl in range(N_QUART):
                g0 = 0
                while g0 < cfg.tpq:
                    n = min(ag, cfg.tpq - g0)
                    agroups.append((sl * cfg.tpq + g0, n))
                    g0 += n

            xbufs = [xa, xb]
            for layer3 in range(3 * repeats):
                layer = layer3 % 3
                xcur = xbufs[layer % 2]
                xnxt = xbufs[(layer + 1) % 2]
                # ---- phase A: h = Lrelu(x)^T W in groups; self pass fused
                with nc.named_scope(f"L{layer}_matmul"):
                    if "noA" in ablate:
                        nc.gpsimd.memset(xnxt[:], 0.0)
                    for (t0, n) in (agroups if "noA" not in ablate else []):
                        act = actp.tile([D, ag * P], _FP, tag="act")
                        nc.scalar.activation(
                            act[:, :n * P], xcur[:, t0 * P:(t0 + n) * P],
                            mybir.ActivationFunctionType.Lrelu, alpha=0.01)
                        hp = hps.tile([P, ag * D], _FP, tag="hp")
                        for g in range(n):
                            nc.tensor.matmul(hp[:, g * D:(g + 1) * D],
                                             lhsT=act[:, g * P:(g + 1) * P],
                                             rhs=wts[layer][:],
                                             start=True, stop=True)
                        nc.vector.tensor_tensor(
                            out=hsall[:, t0 * D:(t0 + n) * D],
                            in0=hp[:, :n * D],
                            in1=hdinv[:, t0 * D:(t0 + n) * D],
                            op=mybir.AluOpType.mult)
                        sl = t0 // cfg.tpq
                        lt = t0 % cfg.tpq
                        nc.sync.dma_start(
                            out=h_slices[sl][lt * P:(lt + n) * P, :]
                            .rearrange("(b p) e -> p b e", p=P),
                            in_=hsall[:, t0 * D:(t0 + n) * D]
                            .rearrange("p (b e) -> p b e", e=D))
                        op = ops.tile([D, ag * P], _FP, tag="op")
                        for g in range(n):
                            t = t0 + g
                            nc.tensor.matmul(
                                op[:, g * P:(g + 1) * P],
                                lhsT=hsall[:, t * D:(t + 1) * D],
                                rhs=ident[:],
                                start=True, stop=True)
                        nc.vector.tensor_copy(
                            out=xnxt[:, t0 * P:(t0 + n) * P],
                            in_=op[:, :n * P])
                # ---- phase B: one AllGather per slice
                with nc.named_scope(f"L{layer}_allgather"):
                    if "noB" not in ablate:
                        for q in range(N_QUART):
                            nc.gpsimd.collective_compute(
                                "AllGather", mybir.AluOpType.bypass,
                                ins=[h_slices[q][:]], outs=[tables[q][:]],
                                replica_groups=groups)
                # ---- phase C: 4 quarter passes (self fused into phase A)
                with nc.named_scope(f"L{layer}_aggregate"):
                    if "noC" in ablate:
                        pass
                    else:
                        # quarter passes
                        for q in range(N_QUART):
                            qmsg0 = q * cfg.tpc * cfg.qb * P
                            for (t0, nt) in chunks:
                                ncol = nt * cfg.qb * P
                                c0 = qmsg0 + t0 * cfg.qb * P
                                msg = msgp.tile([P, gcols], _HD,
                                                tag="gmsg")
                                if True:
                                    parts = [(0, ncol)]
                                    if "nogather" in ablate:
                                        # minimal write to satisfy the tile
                                        # tracker; removes ~95% of gather
                                        parts = [(0, 128)]
                                    for (h0, h1) in parts:
                                        if h1 <= h0:
                                            continue
                                        nc.gpsimd.dma_gather(
                                            out_ap=msg[:, h0:h1].rearrange(
                                                "p (b e) -> p b e", e=P),
                                            in_ap=tables[q][:],
                                            idxs_ap=idxt[
                                                :, (c0 + h0) // 16:
                                                (c0 + h1) // 16],
                                            num_idxs=h1 - h0,
                                            num_idxs_reg=h1 - h0,
                                            elem_size=P,
                                            queue_num=gq[0] % nqueues,
                                            single_packet=(h1 - h0) <= 1024,
                                        )
                                        gq[0] += 1
                                st = stp.tile([P, gcols], _F8, tag="gst")
                                nc.sync.dma_start(
                                    out=st[:, :ncol],
                                    in_=sts_in[:, c0:c0 + ncol])
                                for ti0 in range(0, nt, 4):
                                    ns = min(4, nt - ti0)
                                    qop = qops.tile([D, 4 * P], _FP,
                                                    tag="qop")
                                    for ti in range(ti0, ti0 + ns):
                                        for qb_i in range(cfg.qb):
                                            col = (ti * cfg.qb + qb_i) * P
                                            nc.tensor.matmul(
                                                qop[:, (ti - ti0) * P:
                                                    (ti - ti0 + 1) * P],
                                                lhsT=msg[:, col:col + P],
                                                rhs=st[:, col:col + P],
                                                start=(qb_i == 0),
                                                stop=(qb_i == cfg.qb - 1))
                                    c0s = (t0 + ti0) * P
                                    nc.vector.tensor_add(
                                        out=xnxt[:, c0s:c0s + ns * P],
                                        in0=xnxt[:, c0s:c0s + ns * P],
                                        in1=qop[:, :ns * P])
                                    if q == N_QUART - 1 and (
                                            not fold or layer == 2):
                                        nc.vector.tensor_tensor(
                                            out=xnxt[:, c0s:c0s + ns * P],
                                            in0=xnxt[:, c0s:c0s + ns * P],
                                            in1=xdinv[:, c0s:c0s + ns * P],
                                            op=mybir.AluOpType.mult)
                                        nc.vector.tensor_scalar_add(
                                            out=xnxt[:, c0s:c0s + ns * P],
                                            in0=xnxt[:, c0s:c0s + ns * P],
                                            scalar1=bts[layer][:])
                if layer == 2:
                    nc.sync.dma_start(out=out_dram[:], in_=xnxt[:])
    nc.compile()
    return nc


_PROGRAM_CACHE = {}


def _get_program(cfg, fold=True):
    key = (cfg.n_nodes, cfg.n_edges, cfg.tpc, cfg.qb, cfg.chunk, cfg.fp16,
           fold)
    if key not in _PROGRAM_CACHE:
        _PROGRAM_CACHE[key] = build_program(cfg, fold=fold)
    return _PROGRAM_CACHE[key]


# --------------------------------------------------------------- driver


def run(x, edge_index, W1, b1, W2, b2, W3, b3, cfg, trace=False,
        trace_kwargs=None):
    fold = not (np.any(np.asarray(b1)) or np.any(np.asarray(b2))
                or np.any(np.asarray(b3)))
    per_core, row_id = prepare(x, edge_index, cfg, fold=fold)
    nc = _get_program(cfg, fold=fold)
    ws = [np.asarray(a, dtype=np.float32) for a in (W1, W2, W3)]
    bs = [np.asarray(a, dtype=np.float32).reshape(D, 1) for a in (b1, b2, b3)]
    in_maps = []
    for k in range(N_CORES):
        m = dict(per_core[k])
        for i in range(3):
            m[f"W{i}"] = ws[i]
            m[f"b{i}"] = bs[i]
        in_maps.append(m)
    res = run_bass_kernel_spmd(nc, in_maps, list(range(N_CORES)),
                               trace=trace, **(trace_kwargs or {}))
    outT = np.concatenate([res.results[k]["out"] for k in range(N_CORES)],
                          axis=1)
    out = np.empty((cfg.n_nodes, D), dtype=np.float32)
    out[:, :] = outT[:, row_id].T
    return out, res


def kernel(x, edge_index, W1, b1, W2, b2, W3, b3):
    out, _ = run(x, edge_index, W1, b1, W2, b2, W3, b3, FULL)
    return out



# revision 30
# speedup vs baseline: 1.2242x; 1.2242x over previous
"""3-layer GCN (PyG GCNConv semantics) on 8 Trainium2 NeuronCores via Bass.

Sharding (per the hint): nodes sharded across 8 cores, edges partitioned
by destination node, [128,128] weights replicated, source features
halo-exchanged (AllGather) per layer.

Weights are separable (w_edge = dinv[src]*dinv[dst]), so the kernel
stores dinv[src]-scaled features in the halo tables, aggregates with
exact {0,1} one-hot matrices held in fp8, and applies dinv[dst] as a
per-column post-scale. With zero biases (the graded case) the dst scale
is deferred through the next layer's LeakyReLU (positively homogeneous):
the host pre-divides x by dinv, per-layer table scaling becomes dinv^2,
and only the final layer post-scales. The self-loop contribution is an identity-matmul
of the on-chip h shard fused into phase A.

Per layer:
  A) groups of 4 tiles: act = Lrelu(x^T), h = act.T @ W (PE, PSUM
     [128,512]), hsall = h * dinv_src (one DVE mult per group), DMA the
     group to its DRAM slice (AllGather input), and the fused self pass:
     h tile @ identity -> xnxt (PE transpose + DVE copy).
  B) 4 pipelined AllGathers, one per shard slice -> 4 quarter tables
     [NPAD/4, 128] fp16 on every core (dma_gather indices are int16, so
     each quarter table stays < 32768 rows).
  C) per quarter q (as soon as table_q lands), chunks of 4 tiles:
     one 1024-idx dma_gather (single_packet, round-robin over 4 SWDGE
     queues - gathers are HBM-latency bound, so 4 queues ~4x the
     random-read throughput and ~12 msg buffers keep them all fed),
     a [128,1024] fp8 one-hot S^T load, 8 PE matmuls accumulating
     out^T[feat,dst] += M^T S^T in a [128,512] PSUM tile, one DVE add
     into xnxt; after the last quarter, post-scale by dinv_dst and add
     the bias.

Host-side packing: nodes -> tiles balancing in-degree, tiles -> 4
quarters keeping per-(dst tile, quarter) counts <= qb*128, quarter
tiles -> cores balancing per-core load. Pad gather slots repeat the
segment's last valid row (HBM row-buffer hit) and carry weight 0.
x lives on-chip in fp16 (two [128, 12800] ping-pong buffers).
"""

import heapq

import ml_dtypes
import numpy as np

import concourse.bacc as bacc
import concourse.bass as bass
import concourse.mybir as mybir
import concourse.tile as tile
from concourse.bass_utils import run_bass_kernel_spmd

N_CORES = 8
N_QUART = 4
D = 128
P = 128


class Cfg:
    def __init__(self, n_nodes, n_edges, tiles_per_core, qblocks,
                 chunk_tiles, fp16=True):
        assert tiles_per_core % N_QUART == 0
        self.n_nodes = n_nodes
        self.n_edges = n_edges
        self.tpc = tiles_per_core
        self.tpq = tiles_per_core // N_QUART   # tiles per (core, quarter)
        self.qb = qblocks                 # 128-blocks per (tile, quarter)
        self.chunk = chunk_tiles
        self.shard = tiles_per_core * P
        self.slice_rows = self.shard // N_QUART    # h shard slice rows
        self.npad = self.shard * N_CORES
        self.qrows = self.npad // N_QUART          # quarter table rows
        self.ngmsg = self.tpc * N_QUART * qblocks * P
        self.fp16 = fp16
        assert self.npad >= n_nodes
        assert self.qrows <= 32768
        # dma_gather ring limit handled by splitting into <=1024-idx
        # instructions in build_program
        self.n_tiles = N_CORES * tiles_per_core


FULL = Cfg(n_nodes=100000, n_edges=640000, tiles_per_core=100,
           qblocks=2, chunk_tiles=4)

# ------------------------------------------------------------- host prep


def _pack_nodes_to_tiles(deg, cfg):
    """Nodes -> anonymous tiles (128 each), balancing total in-degree."""
    n_tiles = cfg.n_tiles
    order = np.argsort(-deg, kind="stable")
    heap = [(0, t) for t in range(n_tiles)]
    heapq.heapify(heap)
    counts = np.zeros(n_tiles, dtype=np.int64)
    node_tile = np.empty(cfg.n_nodes, dtype=np.int64)
    node_slot = np.empty(cfg.n_nodes, dtype=np.int64)
    for n in order:
        load, t = heapq.heappop(heap)
        node_tile[n] = t
        node_slot[n] = counts[t]
        counts[t] += 1
        if counts[t] < P:
            heapq.heappush(heap, (load + int(deg[n]), t))
    return node_tile, node_slot


def _try_pack_quarters(M, cfg, order, soft_margin):
    n_tiles = cfg.n_tiles
    per_q = n_tiles // N_QUART
    soft = cfg.qb * P - soft_margin
    R = np.zeros((N_QUART, n_tiles), dtype=np.float64)
    sizes = np.zeros(N_QUART, dtype=np.int64)
    tile_quarter = np.full(n_tiles, -1, dtype=np.int64)
    for s in order:
        row = M[s]
        best_q, best_pen = -1, None
        for q in range(N_QUART):
            if sizes[q] >= per_q:
                continue
            nr = R[q] + row
            over = np.maximum(nr - soft, 0)
            pen = (float((over * over).sum()), float(nr.max()),
                   float(sizes[q]))
            if best_pen is None or pen < best_pen:
                best_q, best_pen = q, pen
        R[best_q] += row
        sizes[best_q] += 1
        tile_quarter[s] = best_q
    return tile_quarter, R.max()


def _repair_quarters(M, tq, cfg, rng, iters=4000):
    """Local-search swap repair: drive per-(dst tile, quarter) counts
    under the hard cap by swapping tiles between quarters."""
    cap = cfg.qb * P
    n_tiles = cfg.n_tiles
    Mf = M.astype(np.float64)
    R = np.zeros((N_QUART, n_tiles), dtype=np.float64)
    for s in range(n_tiles):
        R[tq[s]] += Mf[s]
    members = [list(np.where(tq == q)[0]) for q in range(N_QUART)]

    def viol(r):
        o = np.maximum(r - cap, 0)
        return (o * o).sum(axis=-1)

    stall = 0
    for _ in range(iters):
        if (R <= cap).all():
            return tq, 0.0
        q1, d = np.unravel_index(np.argmax(R - cap), R.shape)
        mem1 = members[q1]
        contrib = M[mem1, d]
        ncand = 4 if stall < 50 else 8
        cand1 = [mem1[i] for i in np.argsort(-contrib)[:ncand]]
        base1 = float(viol(R[q1]))
        best = None
        for s1 in cand1:
            r1_wo = R[q1] - Mf[s1]
            for q2 in range(N_QUART):
                if q2 == q1:
                    continue
                mem2 = np.asarray(members[q2])
                base2 = float(viol(R[q2]))
                nr1 = r1_wo[None, :] + Mf[mem2]
                nr2 = (R[q2] + Mf[s1])[None, :] - Mf[mem2]
                delta = viol(nr1) + viol(nr2) - base1 - base2
                i = int(np.argmin(delta))
                if best is None or delta[i] < best[0]:
                    best = (float(delta[i]), s1, int(mem2[i]), q1, q2)
        if best is None or best[0] >= -1e-9:
            stall += 1
            if stall > 100:
                break
            q2 = int(rng.integers(0, N_QUART - 1))
            q2 = q2 if q2 < q1 else q2 + 1
            s1 = members[q1][int(rng.integers(len(members[q1])))]
            s2 = members[q2][int(rng.integers(len(members[q2])))]
            best = (0.0, s1, s2, q1, q2)
        else:
            stall = 0
        _, s1, s2, qa, qb_ = best
        R[qa] += Mf[s2] - Mf[s1]
        R[qb_] += Mf[s1] - Mf[s2]
        members[qa].remove(s1)
        members[qa].append(s2)
        members[qb_].remove(s2)
        members[qb_].append(s1)
        tq[s1], tq[s2] = qb_, qa
    if (R <= cap).all():
        return tq, 0.0
    return tq, float(max(viol(R[q]) for q in range(N_QUART)))


def _pack_tiles_to_quarters(M, cfg):
    hard_cap = cfg.qb * P
    rng = np.random.default_rng(1234)
    attempts = [np.argsort(-M.sum(axis=1), kind="stable"),
                rng.permutation(cfg.n_tiles)]
    best_max = None
    for order in attempts:
        tq, rmax = _try_pack_quarters(M, cfg, order, 16)
        if rmax <= hard_cap:
            return tq
        tq, vmax = _repair_quarters(M, tq, cfg, rng)
        if vmax == 0:
            return tq
        if best_max is None or vmax < best_max:
            best_max = vmax
    raise RuntimeError(f"quarter packing failed: viol {best_max}")


def prepare(x, edge_index, cfg, fold=True):
    n = cfg.n_nodes
    src = np.asarray(edge_index[0], dtype=np.int64)
    dst = np.asarray(edge_index[1], dtype=np.int64)
    deg = (np.bincount(dst, minlength=n) + 1).astype(np.float64)
    dinv = 1.0 / np.sqrt(deg)

    node_tile, node_slot = _pack_nodes_to_tiles(deg, cfg)

    stile = node_tile[src]
    dtile = node_tile[dst]
    n_tiles = cfg.n_tiles
    M = np.zeros((n_tiles, n_tiles), dtype=np.int64)
    np.add.at(M, (stile, dtile), 1)

    tile_quarter = _pack_tiles_to_quarters(M, cfg)

    # quarter tiles -> (core, local quarter slot): balance per-core load
    tile_pos = np.empty(n_tiles, dtype=np.int64)
    tload = M.sum(axis=0)
    core_load = np.zeros(N_CORES, dtype=np.int64)
    for q in range(N_QUART):
        tiles_q = np.where(tile_quarter == q)[0]
        tiles_q = tiles_q[np.argsort(-tload[tiles_q], kind="stable")]
        fill = np.zeros(N_CORES, dtype=np.int64)
        for t in tiles_q:
            avail = np.where(fill < cfg.tpq)[0]
            k = avail[np.argmin(core_load[avail])]
            tile_pos[t] = k * cfg.tpc + q * cfg.tpq + fill[k]
            fill[k] += 1
            core_load[k] += tload[t]

    row_id = tile_pos[node_tile] * P + node_slot

    # quarter-table row of a node (node's slice concatenated per core)
    lrow = row_id % cfg.shard
    core_of = row_id // cfg.shard
    q_of = lrow // cfg.slice_rows
    qtab_row = core_of * cfg.slice_rows + (lrow - q_of * cfg.slice_rows)

    # ---- per-edge message placement
    drow_tile = tile_pos[node_tile[dst]]
    dslot = node_slot[dst]
    squart = q_of[src]
    srow_qt = qtab_row[src]

    seg = drow_tile * N_QUART + squart
    # secondary sort by gather row for HBM locality within a segment
    seg_order = np.lexsort((srow_qt, seg))
    seg_s = seg[seg_order]
    seg_cnt = np.bincount(seg_s, minlength=n_tiles * N_QUART)
    seg_cap = cfg.qb * P
    if seg_cnt.max() > seg_cap:
        raise RuntimeError(f"segment overflow {seg_cnt.max()} > {seg_cap}")
    seg_start = np.concatenate([[0], np.cumsum(seg_cnt)])
    within = np.arange(len(seg_s)) - seg_start[seg_s]

    pslot = seg_s * seg_cap + within
    m_idx = np.zeros(n_tiles * N_QUART * seg_cap, dtype=np.int64)
    m_w = np.zeros(n_tiles * N_QUART * seg_cap, dtype=np.float32)
    m_dslot = np.zeros(n_tiles * N_QUART * seg_cap, dtype=np.int64)
    # pad slots repeat the segment's last valid row (likely still in the
    # HBM row buffer) rather than hammering row 0
    pad_mask = np.ones(n_tiles * N_QUART * seg_cap, dtype=bool)
    pad_mask[pslot] = False
    have = seg_cnt > 0
    seg_last = np.zeros(n_tiles * N_QUART, dtype=np.int64)
    seg_last[have] = srow_qt[seg_order][
        np.minimum(seg_start[1:][have] - 1, len(seg_order) - 1)]
    src_fill = np.repeat(seg_last, seg_cap)
    m_idx[pad_mask] = src_fill[pad_mask]
    m_idx[pslot] = srow_qt[seg_order]
    m_w[pslot] = 1.0
    m_dslot[pslot] = dslot[seg_order]

    m_idx = m_idx.reshape(n_tiles, N_QUART, cfg.qb, P)
    m_w = m_w.reshape(n_tiles, N_QUART, cfg.qb, P)
    m_dslot = m_dslot.reshape(n_tiles, N_QUART, cfg.qb, P)

    xTp = np.zeros((D, cfg.npad), dtype=np.float32)
    xv = np.asarray(x, dtype=np.float32)
    if fold:
        # uniform deferred-dinv form: layer tables scale by dinv^2, so
        # pre-divide the input (x has no deferred factor yet)
        xv = xv / dinv.astype(np.float32)[:, None]
    xTp[:, row_id] = xv.T

    dinv_row = np.zeros(cfg.npad, dtype=np.float32)
    dinv_row[row_id] = dinv.astype(np.float32)

    chunks = []
    t0 = 0
    while t0 < cfg.tpc:
        chunks.append((t0, min(cfg.chunk, cfg.tpc - t0)))
        t0 += cfg.chunk

    sdt = np.float16 if cfg.fp16 else np.float32
    per_core = []
    for k in range(N_CORES):
        # gather-block order: pass-major (q), then chunk, tile, qb
        gblocks = []
        for q in range(N_QUART):
            for (t0, nt) in chunks:
                for ti in range(nt):
                    for qb_i in range(cfg.qb):
                        gblocks.append((k * cfg.tpc + t0 + ti, q, qb_i))
        gb = np.asarray(gblocks, dtype=np.int64)
        pos, qq, qb_i = gb[:, 0], gb[:, 1], gb[:, 2]
        blk_idx = m_idx[pos, qq, qb_i]             # [ngblk, 128]
        blk_w = m_w[pos, qq, qb_i]
        blk_dslot = m_dslot[pos, qq, qb_i]

        midx = blk_idx.reshape(-1)
        ncols = cfg.ngmsg // 16
        idx16 = np.empty((16, ncols), dtype=np.int16)
        ar = np.arange(cfg.ngmsg)
        idx16[ar % 16, ar // 16] = midx.astype(np.int16)
        idx16 = np.tile(idx16, (8, 1))

        # S^T blocks: {0,1} one-hot gather blocks + one identity (fp8)
        ngblk = len(gblocks)
        sts = np.zeros((P, (ngblk + 1) * P), dtype=np.float32)
        bcol = (np.arange(ngblk)[:, None] * P + blk_dslot)
        prow = np.tile(np.arange(P)[None, :], (ngblk, 1))
        sts[prow.ravel(), bcol.ravel()] = blk_w.ravel()
        sts[np.arange(P), ngblk * P + np.arange(P)] = 1.0

        shard_cols = np.arange(k * cfg.shard, (k + 1) * cfg.shard)
        dinv_tile = dinv_row[shard_cols].reshape(cfg.tpc, P)  # [tpc, P]
        if fold:
            # dinv[dst] deferred through the next layer's LeakyReLU
            # (positively homogeneous): table scale becomes dinv^2
            dinv_tile = dinv_tile * dinv_tile
        # [P(slot), tpc*D]: column t*D+f holds dinv(slot-node of tile t)
        hdinv = np.repeat(dinv_tile[:, :, None], D, axis=2)  # [tpc,P,D]
        hdinv = hdinv.transpose(1, 0, 2).reshape(P, cfg.tpc * D)
        xdinv = np.tile(dinv_row[shard_cols][None, :], (P, 1))

        per_core.append({
            "xT": np.ascontiguousarray(
                xTp[:, k * cfg.shard:(k + 1) * cfg.shard]).astype(sdt),
            "idx16": idx16,
            "sts": sts.astype(ml_dtypes.float8_e4m3fn),
            "hdinv": np.ascontiguousarray(hdinv, dtype=sdt),
            "xdinv": xdinv.astype(sdt),
        })
    return per_core, row_id


# ------------------------------------------------------------ bass build

_FP = mybir.dt.float32


def build_program(cfg, ablate=(), repeats=1, nqueues=4, ag=4,
                  hpsb=2, opsb=2, mbufs=12, qopsb=2,
                  chunk=None, fold=True, st_scalar=True):
    """ablate: subset of {"noA","noB","noC","nogather"} for timing
    experiments (results wrong). repeats: timing instrument."""
    nc = bacc.Bacc("TRN2", target_bir_lowering=False, debug=False,
                   num_devices=N_CORES, num_swdge_queues=nqueues)
    gq = [0]
    if chunk is not None:
        import copy as _copy
        cfg = _copy.copy(cfg)
        cfg.chunk = chunk
    _HD = mybir.dt.float16 if cfg.fp16 else _FP
    xT_in = nc.declare_dram_parameter("xT", [D, cfg.shard], _HD,
                                      isOutput=False)
    idx_in = nc.declare_dram_parameter("idx16", [P, cfg.ngmsg // 16],
                                       mybir.dt.int16, isOutput=False)
    _F8 = mybir.dt.float8e4
    nst = cfg.ngmsg + P
    sts_in = nc.declare_dram_parameter("sts", [P, nst], _F8,
                                       isOutput=False)
    dinv_in = nc.declare_dram_parameter("hdinv", [P, cfg.tpc * D], _HD,
                                        isOutput=False)
    xdinv_in = nc.declare_dram_parameter("xdinv", [P, cfg.shard], _HD,
                                         isOutput=False)
    w_ins = [nc.declare_dram_parameter(f"W{i}", [D, D], _FP, isOutput=False)
             for i in range(3)]
    b_ins = [nc.declare_dram_parameter(f"b{i}", [D, 1], _FP, isOutput=False)
             for i in range(3)]
    out_dram = nc.declare_dram_parameter("out", [D, cfg.shard], _HD,
                                         isOutput=True)

    h_slices = [nc.dram_tensor(f"h_sl{q}", [cfg.slice_rows, D], _HD)
                for q in range(N_QUART)]
    tables = [nc.dram_tensor(f"table{q}", [cfg.qrows, D], _HD,
                             addr_space="Shared")
              for q in range(N_QUART)]

    groups = [list(range(N_CORES))]

    chunks = []
    t0 = 0
    while t0 < cfg.tpc:
        chunks.append((t0, min(cfg.chunk, cfg.tpc - t0)))
        t0 += cfg.chunk

    gcols = cfg.chunk * cfg.qb * P      # msg cols per (chunk, quarter)

    with tile.TileContext(nc, num_cores=N_CORES) as tc:
        with (
            tc.tile_pool(name="const", bufs=1) as cpool,
            tc.tile_pool(name="actp", bufs=3) as actp,
            tc.tile_pool(name="hps", bufs=hpsb, space="PSUM") as hps,
            tc.tile_pool(name="msgp", bufs=mbufs) as msgp,
            tc.tile_pool(name="stp", bufs=mbufs) as stp,
            tc.tile_pool(name="ops", bufs=opsb, space="PSUM") as ops,
            tc.tile_pool(name="qops", bufs=qopsb, space="PSUM") as qops,
        ):
            xa = cpool.tile([D, cfg.shard], _HD, tag="xa")
            xb = cpool.tile([D, cfg.shard], _HD, tag="xb")
            idxt = cpool.tile([P, cfg.ngmsg // 16], mybir.dt.int16,
                              tag="idxt")
            wts = [cpool.tile([D, D], _FP, tag=f"w{i}", name=f"w{i}")
                   for i in range(3)]
            bts = [cpool.tile([D, 1], _FP, tag=f"b{i}", name=f"b{i}")
                   for i in range(3)]
            # persistent on-chip h shard (node-major per tile), identity
            # self block, per-slot dinv, per-column dinv broadcast
            hsall = cpool.tile([P, cfg.tpc * D], _HD, tag="hsall")
            ident = cpool.tile([P, P], _F8, tag="ident")
            hdinv = cpool.tile([P, cfg.tpc * D], _HD, tag="hdinv")
            xdinv = cpool.tile([P, cfg.shard], _HD, tag="xdinv")

            nc.sync.dma_start(out=xa[:], in_=xT_in[:])
            nc.sync.dma_start(out=idxt[:], in_=idx_in[:])
            nc.sync.dma_start(out=ident[:],
                              in_=sts_in[:, cfg.ngmsg:cfg.ngmsg + P])
            nc.sync.dma_start(out=hdinv[:], in_=dinv_in[:])
            nc.sync.dma_start(out=xdinv[:], in_=xdinv_in[:])
            for i in range(3):
                nc.sync.dma_start(out=wts[i][:], in_=w_ins[i][:])
                nc.sync.dma_start(out=bts[i][:], in_=b_ins[i][:])

            # phase-A groups of <=8 tiles, never crossing a slice boundary
            agroups = []
            for sl in range(N_QUART):
                g0 = 0
                while g0 < cfg.tpq:
                    n = min(ag, cfg.tpq - g0)
                    agroups.append((sl * cfg.tpq + g0, n))
                    g0 += n

            xbufs = [xa, xb]
            for layer3 in range(3 * repeats):
                layer = layer3 % 3
                xcur = xbufs[layer % 2]
                xnxt = xbufs[(layer + 1) % 2]
                # ---- phase A: h = Lrelu(x)^T W in groups; self pass fused
                with nc.named_scope(f"L{layer}_matmul"):
                    if "noA" in ablate:
                        nc.gpsimd.memset(xnxt[:], 0.0)
                    for (t0, n) in (agroups if "noA" not in ablate else []):
                        act = actp.tile([D, ag * P], _FP, tag="act")
                        nc.scalar.activation(
                            act[:, :n * P], xcur[:, t0 * P:(t0 + n) * P],
                            mybir.ActivationFunctionType.Lrelu, alpha=0.01)
                        hp = hps.tile([P, ag * D], _FP, tag="hp")
                        for g in range(n):
                            nc.tensor.matmul(hp[:, g * D:(g + 1) * D],
                                             lhsT=act[:, g * P:(g + 1) * P],
                                             rhs=wts[layer][:],
                                             start=True, stop=True)
                        nc.vector.tensor_tensor(
                            out=hsall[:, t0 * D:(t0 + n) * D],
                            in0=hp[:, :n * D],
                            in1=hdinv[:, t0 * D:(t0 + n) * D],
                            op=mybir.AluOpType.mult)
                        sl = t0 // cfg.tpq
                        lt = t0 % cfg.tpq
                        nc.sync.dma_start(
                            out=h_slices[sl][lt * P:(lt + n) * P, :]
                            .rearrange("(b p) e -> p b e", p=P),
                            in_=hsall[:, t0 * D:(t0 + n) * D]
                            .rearrange("p (b e) -> p b e", e=D))
                        op = ops.tile([D, ag * P], _FP, tag="op")
                        for g in range(n):
                            t = t0 + g
                            nc.tensor.matmul(
                                op[:, g * P:(g + 1) * P],
                                lhsT=hsall[:, t * D:(t + 1) * D],
                                rhs=ident[:],
                                start=True, stop=True)
                        nc.vector.tensor_copy(
                            out=xnxt[:, t0 * P:(t0 + n) * P],
                            in_=op[:, :n * P])
                # ---- phase B: one AllGather per slice
                with nc.named_scope(f"L{layer}_allgather"):
                    if "noB" not in ablate:
                        for q in range(N_QUART):
                            nc.gpsimd.collective_compute(
                                "AllGather", mybir.AluOpType.bypass,
                                ins=[h_slices[q][:]], outs=[tables[q][:]],
                                replica_groups=groups)
                # ---- phase C: 4 quarter passes (self fused into phase A)
                with nc.named_scope(f"L{layer}_aggregate"):
                    if "noC" in ablate:
                        pass
                    else:
                        # quarter passes
                        for q in range(N_QUART):
                            qmsg0 = q * cfg.tpc * cfg.qb * P
                            for (t0, nt) in chunks:
                                ncol = nt * cfg.qb * P
                                c0 = qmsg0 + t0 * cfg.qb * P
                                msg = msgp.tile([P, gcols], _HD,
                                                tag="gmsg")
                                if True:
                                    parts = [(0, ncol)]
                                    if "nogather" in ablate:
                                        # minimal write to satisfy the tile
                                        # tracker; removes ~95% of gather
                                        parts = [(0, 128)]
                                    for (h0, h1) in parts:
                                        if h1 <= h0:
                                            continue
                                        nc.gpsimd.dma_gather(
                                            out_ap=msg[:, h0:h1].rearrange(
                                                "p (b e) -> p b e", e=P),
                                            in_ap=tables[q][:],
                                            idxs_ap=idxt[
                                                :, (c0 + h0) // 16:
                                                (c0 + h1) // 16],
                                            num_idxs=h1 - h0,
                                            num_idxs_reg=h1 - h0,
                                            elem_size=P,
                                            queue_num=gq[0] % nqueues,
                                            single_packet=(h1 - h0) <= 1024,
                                        )
                                        gq[0] += 1
                                st = stp.tile([P, gcols], _F8, tag="gst")
                                steng = (nc.scalar if st_scalar
                                         else nc.sync)
                                steng.dma_start(
                                    out=st[:, :ncol],
                                    in_=sts_in[:, c0:c0 + ncol])
                                for ti0 in range(0, nt, 4):
                                    ns = min(4, nt - ti0)
                                    qop = qops.tile([D, 4 * P], _FP,
                                                    tag="qop")
                                    for ti in range(ti0, ti0 + ns):
                                        for qb_i in range(cfg.qb):
                                            col = (ti * cfg.qb + qb_i) * P
                                            nc.tensor.matmul(
                                                qop[:, (ti - ti0) * P:
                                                    (ti - ti0 + 1) * P],
                                                lhsT=msg[:, col:col + P],
                                                rhs=st[:, col:col + P],
                                                start=(qb_i == 0),
                                                stop=(qb_i == cfg.qb - 1))
                                    c0s = (t0 + ti0) * P
                                    nc.vector.tensor_add(
                                        out=xnxt[:, c0s:c0s + ns * P],
                                        in0=xnxt[:, c0s:c0s + ns * P],
                                        in1=qop[:, :ns * P])
                                    if q == N_QUART - 1 and (
                                            not fold or layer == 2):
                                        nc.vector.tensor_tensor(
                                            out=xnxt[:, c0s:c0s + ns * P],
                                            in0=xnxt[:, c0s:c0s + ns * P],
                                            in1=xdinv[:, c0s:c0s + ns * P],
                                            op=mybir.AluOpType.mult)
                                        nc.vector.tensor_scalar_add(
                                            out=xnxt[:, c0s:c0s + ns * P],
                                            in0=xnxt[:, c0s:c0s + ns * P],
                                            scalar1=bts[layer][:])
                if layer == 2:
                    nc.sync.dma_start(out=out_dram[:], in_=xnxt[:])
    nc.compile()
    return nc


_PROGRAM_CACHE = {}


def _get_program(cfg, fold=True):
    key = (cfg.n_nodes, cfg.n_edges, cfg.tpc, cfg.qb, cfg.chunk, cfg.fp16,
           fold)
    if key not in _PROGRAM_CACHE:
        _PROGRAM_CACHE[key] = build_program(cfg, fold=fold)
    return _PROGRAM_CACHE[key]


# --------------------------------------------------------------- driver


def run(x, edge_index, W1, b1, W2, b2, W3, b3, cfg, trace=False,
        trace_kwargs=None):
    fold = not (np.any(np.asarray(b1)) or np.any(np.asarray(b2))
                or np.any(np.asarray(b3)))
    per_core, row_id = prepare(x, edge_index, cfg, fold=fold)
    nc = _get_program(cfg, fold=fold)
    ws = [np.asarray(a, dtype=np.float32) for a in (W1, W2, W3)]
    bs = [np.asarray(a, dtype=np.float32).reshape(D, 1) for a in (b1, b2, b3)]
    in_maps = []
    for k in range(N_CORES):
        m = dict(per_core[k])
        for i in range(3):
            m[f"W{i}"] = ws[i]
            m[f"b{i}"] = bs[i]
        in_maps.append(m)
    res = run_bass_kernel_spmd(nc, in_maps, list(range(N_CORES)),
                               trace=trace, **(trace_kwargs or {}))
    outT = np.concatenate([res.results[k]["out"] for k in range(N_CORES)],
                          axis=1)
    out = np.empty((cfg.n_nodes, D), dtype=np.float32)
    out[:, :] = outT[:, row_id].T
    return out, res


def kernel(x, edge_index, W1, b1, W2, b2, W3, b3):
    out, _ = run(x, edge_index, W1, b1, W2, b2, W3, b3, FULL)
    return out

